# revision 64
# baseline (speedup 1.0000x reference)
"""MinGRU block kernel for 8 TRN2 NeuronCores.

Sharding: core c -> (batch b = c//2, T-half = c%2).  Each core processes
4096 rows of (T=8192) for one batch plus a 128-row scan warmup prefix.
The minGRU recurrence h_t = (1-z_t) h_{t-1} + z_t g_t is evaluated in
linear space with the DVE TensorTensorScan instruction (state fp32);
the warmup prefix exploits exponential forgetting (prod(1-z) < e^-30
over 128 steps) so no cross-core communication is needed: the half=1
core recomputes its predecessor's last 128 rows, the half=0 core scans
128 masked dummy rows and blends its true initial state (0.5) instead.

Engine placement per 512-row chunk (trace-tuned):
  PE   : gate-z + FFN2 matmuls in fp8e4 DoubleRow (2 accum steps of
         K=256), gate-p + FFN1 in fp16 (accuracy), plus rank-1 b2-bias
         and rank-128 identity matmuls that add the residual in PSUM.
  DVE  : bn_stats/aggr + rsqrt Newton seed, LN affine applies, a=1-z,
         g=max(p+bh+.5,s), b=g*z, xn residual add, the 4 scans.
  ACT  : sigmoids, relu, the uT fp16->fp8 cast, psY f32->f16 drain.
  SP   : every DMA: x loads, uT/u2T/hT xbar transposes, out stores.
  Pool : nothing — the TRN2 Pool engine cannot touch PSUM and lacks
         TensorScalarPtr, and its 0.42-efficiency TensorTensor ops
         lose more in scan-chain latency than they save on DVE.
PSUM: 4 banks gate psums, 4 banks FFN1/FFN2+residual (shared ring).
LayerNorm gains/biases are folded into the weight matrices host-side;
x is fp16 host-side (halves input DMA); out is stored f16 and upcast
on the host (error budget is 2e-2 of max|out| ~ 0.196 abs).
"""

import numpy as np

B, T, H = 4, 8192, 512
LN_EPS = 1e-5
HALF_T = T // 2          # rows per core (output)
WARM = 128               # scan warmup rows
ROWS = HALF_T + WARM     # input rows per core
N_CORES = 8
CHUNK = 512              # rows per pipeline chunk
N_CHUNKS = HALF_T // CHUNK
HC = H // 128            # 4 H-chunks

# precision config: each fp8 matmul path adds ~0.13-0.15 max-abs error of
# the 0.196 budget (2e-2 of max|out|), combining in quadrature.  fp8 for
# the z-gate (sigmoid compresses 4x) and FFN2 (with host-side bias feedback
# of the mean weight-quantization error) measures ~1.6e-2; Wh and FFN1
# stay fp16.
FP8_WZ = True
FP8_FFN1 = False
FP8_FFN2 = True
SCAN_ON_POOL = 0         # Pool lacks TensorScalarPtr on trn2: scans stay DVE
NEWTON_ITERS = 1

_cache = {}


# ---------------------------------------------------------------------------
# walrus workaround: the compiler in this container caps sync commands per
# instruction at 1 wait + 1 update.  Tile attaches N waits/updates freely;
# split the excess onto same-engine NoOps (before for waits, after for
# updates).
# ---------------------------------------------------------------------------
def _split_excess_waits(nc):
    import bass_rust

    ctr = [0]

    def mknop(engine, waits, updates):
        ctr[0] += 1
        nop = bass_rust.InstNoOp(name=f"splitw-{ctr[0]}")
        nop.engine = engine
        nop.sync_info = bass_rust.SyncInfo(on_wait=list(waits), on_update=list(updates))
        nc.register_instruction(nop)
        return nop

    for f in nc.m.functions:
        for bb in f.blocks:
            insts = list(bb.instructions)
            out = []
            changed = False
            for ins in insts:
                si = ins.sync_info
                if si is None:
                    out.append(ins)
                    continue
                waits = list(si.on_wait or [])
                updates = list(si.on_update or [])
                if len(waits) <= 1 and len(updates) <= 1:
                    out.append(ins)
                    continue
                changed = True
                for w in waits[1:]:
                    out.append(mknop(ins.engine, [w], []))
                si.on_wait = waits[:1]
                si.on_update = updates[:1]
                out.append(ins)
                for u in updates[1:]:
                    out.append(mknop(ins.engine, [], [u]))
            if changed:
                bb.instructions = out


# ---------------------------------------------------------------------------
# kernel builder
# ---------------------------------------------------------------------------
def _build():
    import concourse.bass as bass
    import concourse.tile as tile
    from concourse import mybir

    f32, f16 = mybir.dt.float32, mybir.dt.float16
    f8 = mybir.dt.float8e4
    AF = mybir.ActivationFunctionType
    OP = mybir.AluOpType
    DR = mybir.MatmulPerfMode.DoubleRow

    zdt = f8 if FP8_WZ else f16
    w1dt = f8 if FP8_FFN1 else f16
    w2dt = f8 if FP8_FFN2 else f16

    nc = bass.Bass()
    xs_e = nc.declare_dram_parameter("xs", [ROWS, H], f16, isOutput=False)
    wz_e = nc.declare_dram_parameter("wz", [128, HC * H], zdt, isOutput=False)
    wh_e = nc.declare_dram_parameter("wh", [128, HC * H], f16, isOutput=False)
    w1_e = nc.declare_dram_parameter("w1", [128, HC * H], w1dt, isOutput=False)
    w2_e = nc.declare_dram_parameter("w2", [128, HC * H], w2dt, isOutput=False)
    # packed per-partition scalars: cols 0-3 bz, 4-7 bh, 8-11 bh+0.5,
    # 12-15 b1 (per 128-channel chunk), 16 m (carry mask), 17 c (carry bias)
    mi_e = nc.declare_dram_parameter("mi", [128, 22], f32, isOutput=False)
    b2_e = nc.declare_dram_parameter("b2", [1, H], f16, isOutput=False)
    id_e = nc.declare_dram_parameter("idn", [128, 128], f16, isOutput=False)
    out_e = nc.declare_dram_parameter("out", [HALF_T, H], f16, isOutput=True)

    with tile.TileContext(nc) as tc:
        from contextlib import ExitStack

        with ExitStack() as ctx:
            ep = ctx.enter_context

            const = ep(tc.tile_pool(name="const", bufs=1))
            xp = ep(tc.tile_pool(name="xp", bufs=4))
            up = ep(tc.tile_pool(name="up", bufs=4))
            uTp = ep(tc.tile_pool(name="uTp", bufs=4))
            u8p = ep(tc.tile_pool(name="u8p", bufs=4))
            gp = ep(tc.tile_pool(name="gp", bufs=8))
            hp = ep(tc.tile_pool(name="hp", bufs=10))
            xnp = ep(tc.tile_pool(name="xnp", bufs=4))
            hnp = ep(tc.tile_pool(name="hnp", bufs=3))
            h2p = ep(tc.tile_pool(name="h2p", bufs=4))
            op_ = ep(tc.tile_pool(name="op", bufs=4))
            stp = ep(tc.tile_pool(name="stp", bufs=24))
            # 8 PSUM banks: gates 4, FFN1+FFN2 share 4 (same tile shape);
            # the h-transpose runs on the DMA xbar so no PSUM is needed
            psG = ep(tc.tile_pool(name="psG", bufs=5, space="PSUM"))
            psFY = ep(tc.tile_pool(name="psFY", bufs=3, space="PSUM"))

            # ---- constants ----
            def load_w(name, ext, dt):
                t = const.tile([128, HC * H], dt, name=name, tag=name)
                nc.sync.dma_start(t[:], ext[:])
                # [128, hi, m] view: hi = contraction plane, m = out channel
                return t[:].rearrange("a (k m) -> a k m", k=HC)

            WZ = load_w("wz", wz_e, zdt)
            WH = load_w("wh", wh_e, f16)
            W1 = load_w("w1", w1_e, w1dt)
            W2 = load_w("w2", w2_e, w2dt)

            mi = const.tile([128, 22], f32, name="mi", tag="mi")
            nc.sync.dma_start(mi[:], mi_e[:])
            b2r = const.tile([1, H], f16, name="b2r", tag="b2r")
            nc.sync.dma_start(b2r[:], b2_e[:])
            ones1 = const.tile([1, 128], f16, name="ones1", tag="ones1")
            nc.gpsimd.memset(ones1[:], 1.0)
            idn = const.tile([128, 128], f16, name="idn", tag="idn")
            nc.sync.dma_start(idn[:], id_e[:])

            BZ = [mi[:, j : j + 1] for j in range(0, 4)]
            BH = [mi[:, j : j + 1] for j in range(4, 8)]
            BH05 = [mi[:, j : j + 1] for j in range(8, 12)]
            B1 = [mi[:, j : j + 1] for j in range(12, 16)]
            M_AP = mi[:, 16:17]
            C_AP = mi[:, 17:18]

            i32 = mybir.dt.int32

            def rstd_and_nm(mvall, n, ci, which):
                """mvall [128, 2n] = (means | vars) -> (rstd y, -mu*rstd nm).

                rstd = 1/sqrt(var+eps): q=1/(var+eps) (HW divide), sqrt
                bit-hack seed (i>>1)+0x1fbd1df5, Newton rsqrt steps.
                """
                means, vars_ = mvall[:, 0:n], mvall[:, n : 2 * n]
                ve = stp.tile([128, n], f32, name=f"ve{which}_{ci}", tag="ve")
                nc.vector.tensor_scalar(ve[:], vars_, LN_EPS, None, OP.add)
                q = stp.tile([128, n], f32, name=f"q{which}_{ci}", tag="q")
                nc.vector.reciprocal(q[:], ve[:])
                y = stp.tile([128, n], f32, name=f"y{which}_{ci}", tag="y")
                nc.vector.tensor_scalar(
                    y[:].bitcast(i32), q[:].bitcast(i32), 1, None,
                    OP.logical_shift_right,
                )
                nc.vector.tensor_scalar(
                    y[:].bitcast(i32), y[:].bitcast(i32), 0x1FBD1DF5, None, OP.add
                )
                w = stp.tile([128, n], f32, name=f"w{which}_{ci}", tag="w")
                for _ in range(NEWTON_ITERS):  # y <- y*(1.5 - 0.5*ve*y^2)
                    nc.vector.tensor_mul(w[:], y[:], y[:])
                    nc.vector.tensor_mul(w[:], w[:], ve[:])
                    nc.vector.tensor_scalar(w[:], w[:], -0.5, 1.5, OP.mult, OP.add)
                    nc.vector.tensor_mul(y[:], y[:], w[:])
                nm = stp.tile([128, n], f32, name=f"nm{which}_{ci}", tag="nm")
                nc.vector.scalar_tensor_tensor(nm[:], means, -1.0, y[:], OP.mult, OP.mult)
                return y, nm

            def layernorm_group(src, nsub, ci, which, pool, tag, tbufs=None, on_act=False):
                """standardize rows of src [128, nsub, H] -> f16 tile same shape.

                bn_stats/aggr on DVE for the stats; the affine apply is a
                DVE tensor_scalar (4x fast mode) with per-partition scale
                rstd and bias -mu*rstd.
                """
                n = nsub
                mvall = stp.tile([128, 2 * n], f32, name=f"mv{which}_{ci}", tag="mv")
                for p in range(n):
                    st = stp.tile([128, 6], f32, name=f"bn{which}_{ci}_{p}", tag="bn")
                    nc.vector.bn_stats(st[:], src[:, p, :])
                    # mean -> col p, var -> col n+p  (stride-n pair)
                    nc.vector.bn_aggr(mvall[:, p : p + n + 1 : n], st[:])
                y, nm = rstd_and_nm(mvall[:], n, ci, which)
                ut = pool.tile(
                    [128, nsub, H], f16, name=f"{tag}_{ci}", tag=tag, bufs=tbufs
                )
                for p in range(n):
                    if on_act:
                        nc.scalar.activation(
                            ut[:, p, :], src[:, p, :], AF.Identity,
                            bias=nm[:, p : p + 1], scale=y[:, p : p + 1],
                        )
                    else:
                        nc.vector.tensor_scalar(
                            ut[:, p, :], src[:, p, :],
                            y[:, p : p + 1], nm[:, p : p + 1],
                            OP.mult, OP.add,
                        )
                return ut

            def transpose_cast(ut, nsub, tlen, ci, tag, cast, tbufs=None, eng=None):
                """u [128, nsub, H] f16 -> uT [128, HC, tlen] (+fp8 copy).

                DMA xbar transpose per subtile into an f16 tile, then one
                ACT copy casts the whole tile to fp8 for DoubleRow matmuls.
                """
                tT = uTp.tile(
                    [128, HC, tlen], f16, name=f"{tag}T_{ci}", tag=f"{tag}T", bufs=tbufs
                )
                for p in range(nsub):
                    (eng or nc.sync).dma_start_transpose(
                        tT[:, :, p * 128 : (p + 1) * 128], ut[:, p, :]
                    )
                if not cast:
                    return tT, tT
                t8 = u8p.tile(
                    [128, HC, tlen], f8, name=f"{tag}8_{ci}", tag=f"{tag}8", bufs=tbufs
                )
                nc.scalar.activation(
                    t8[:].rearrange("a k m -> a (k m)"),
                    tT[:].rearrange("a k m -> a (k m)"),
                    AF.Copy,
                )
                return tT, t8

            def mm_accum(ps, W, rhs8, ho, fp8):
                """accumulate ps[:, :tlen] = (W^T @ u)[ho block] over K=512."""
                if fp8:
                    for ki in range(HC // 2):
                        nc.tensor.matmul(
                            ps,
                            W[:, 2 * ki : 2 * ki + 2, ho * 128 : (ho + 1) * 128],
                            rhs8[:, 2 * ki : 2 * ki + 2, :],
                            start=(ki == 0),
                            stop=(ki == HC // 2 - 1),
                            perf_mode=DR,
                        )
                else:
                    for hi in range(HC):
                        nc.tensor.matmul(
                            ps,
                            W[:, hi, ho * 128 : (ho + 1) * 128],
                            rhs8[:, hi, :],
                            start=(hi == 0),
                            stop=(hi == HC - 1),
                        )

            carry = [None] * HC  # AP of [128,1] initial state per H-chunk

            def front(ci):
                warm = ci == 0
                tlen = WARM if warm else CHUNK
                t0 = 0 if warm else WARM + (ci - 1) * CHUNK
                nsub = tlen // 128
                wtag = "w" if warm else ""
                wb = 1 if warm else None

                # ---- stage A: load + LN1 ----
                xt = xp.tile([128, nsub, H], f16, name=f"x_{ci}", tag=f"x{wtag}", bufs=wb)
                nc.sync.dma_start(
                    xt[:],
                    xs_e[t0 : t0 + tlen, :].rearrange("(s p) h -> p s h", p=128),
                )
                ut = layernorm_group(xt, nsub, ci, 1, up, f"u{wtag}", wb)
                uT, u8 = transpose_cast(ut, nsub, tlen, ci, f"u{wtag}", FP8_WZ, wb)

                # ---- stage B+C: gate matmuls, gates ----
                # pT first: its consumer chain (s -> g -> b, through 3
                # engines) is the long pole into the scan; kT -> z -> a is
                # short and also needs the fp8 cast.
                gates = []
                for ho in range(HC):
                    pTt = psG.tile([128, CHUNK], f32, name=f"pT_{ci}_{ho}", tag="psG")
                    pT = pTt[:, :tlen]
                    mm_accum(pT, WH, uT, ho, False)
                    kTt = psG.tile([128, CHUNK], f32, name=f"kT_{ci}_{ho}", tag="psG")
                    kT = kTt[:, :tlen]
                    mm_accum(kT, WZ, u8, ho, FP8_WZ)
                    s = gp.tile([128, tlen], f16, name=f"s_{ci}_{ho}", tag=f"s{wtag}", bufs=wb)
                    nc.scalar.activation(s[:], pT, AF.Sigmoid, bias=BH[ho], scale=1.0)
                    g = gp.tile([128, tlen], f16, name=f"g_{ci}_{ho}", tag=f"g{wtag}", bufs=wb)
                    nc.vector.scalar_tensor_tensor(
                        g[:], pT, BH05[ho], s[:], OP.add, OP.max
                    )
                    z = gp.tile([128, tlen], f16, name=f"z_{ci}_{ho}", tag=f"z{wtag}", bufs=wb)
                    nc.scalar.activation(z[:], kT, AF.Sigmoid, bias=BZ[ho], scale=1.0)
                    a = gp.tile([128, tlen], f16, name=f"a_{ci}_{ho}", tag=f"a{wtag}", bufs=wb)
                    nc.vector.tensor_scalar(a[:], z[:], -1.0, 1.0, OP.mult, OP.add)
                    b = gp.tile([128, tlen], f16, name=f"b_{ci}_{ho}", tag=f"b{wtag}", bufs=wb)
                    nc.vector.tensor_mul(b[:], g[:], z[:])
                    gates.append((a, b))

                return ci, warm, tlen, xt, gates

            def scan_stage(st):
                ci, warm, tlen, xt, gates = st
                wtag = "w" if warm else ""
                wb = 1 if warm else None
                hTs = []
                for ho in range(HC):
                    a, b = gates[ho]
                    hT = hp.tile(
                        [128, tlen], f16, name=f"hT_{ci}_{ho}", tag=f"hT{wtag}", bufs=wb
                    )
                    init = 0.5 if warm else carry[ho]
                    eng = nc.gpsimd if ho >= HC - SCAN_ON_POOL else nc.vector
                    eng.tensor_tensor_scan(
                        hT[:], a[:], b[:], init, OP.mult, OP.add
                    )
                    hTs.append(hT)

                if warm:
                    # blend: init = m * h_warm_end + c   (m=0 -> 0.5, m=1 -> carry)
                    for ho in range(HC):
                        bl = stp.tile([128, 1], f32, name=f"bl_{ho}", tag="bl")
                        nc.vector.scalar_tensor_tensor(
                            bl[:],
                            hTs[ho][:, tlen - 1 : tlen],
                            M_AP,
                            C_AP,
                            OP.mult,
                            OP.add,
                        )
                        carry[ho] = bl[:]
                    return None

                for ho in range(HC):
                    carry[ho] = hTs[ho][:, tlen - 1 : tlen]
                return ci, xt, hTs

            def back_ef(state):
                ci, xt, hTs = state
                tlen = CHUNK
                nsub = tlen // 128

                # ---- stage E: h back to natural (DMA xbar transpose) ----
                hn = hnp.tile([128, nsub, H], f16, name=f"hN_{ci}", tag="hN")
                for hc in range(HC):
                    nc.sync.dma_start_transpose(
                        hn[:].rearrange("a s h -> a s h")[
                            :, :, hc * 128 : (hc + 1) * 128
                        ],
                        hTs[hc][:],
                    )
                xn = xnp.tile([128, nsub, H], f16, name=f"xn_{ci}", tag="xn")
                for p in range(nsub):
                    nc.vector.tensor_add(xn[:, p, :], xt[:, p, :], hn[:, p, :])
                # ---- stage F: LN2 ----
                u2 = layernorm_group(xn, nsub, ci, 2, up, "u2")
                _, u28 = transpose_cast(u2, nsub, tlen, ci, "u2", FP8_FFN1)
                return ci, xn, u28

            def back_gh(state):
                ci, xn, u28 = state
                tlen = CHUNK
                t0 = WARM + (ci - 1) * CHUNK
                nsub = tlen // 128

                # ---- stage G: FFN1 + relu ----
                h2 = h2p.tile([128, HC, tlen], w2dt, name=f"h2_{ci}", tag="h2")
                for hh in range(HC):
                    h1 = psFY.tile([128, tlen], f32, name=f"h1_{ci}_{hh}", tag="psFY")
                    mm_accum(h1[:], W1, u28, hh, FP8_FFN1)
                    nc.scalar.activation(
                        h2[:, hh, :], h1[:], AF.Relu, bias=B1[hh], scale=1.0
                    )

                # ---- stage H: FFN2 + bias + residual (all in PSUM) + store ----
                for p in range(nsub):
                    y = psFY.tile([128, H], f32, name=f"y_{ci}_{p}", tag="psFY")
                    if FP8_FFN2:
                        for ki in range(HC // 2):
                            nc.tensor.matmul(
                                y[:],
                                h2[:, 2 * ki : 2 * ki + 2, p * 128 : (p + 1) * 128],
                                W2[:, 2 * ki : 2 * ki + 2, :],
                                start=(ki == 0),
                                stop=False,
                                perf_mode=DR,
                            )
                    else:
                        for hh in range(HC):
                            nc.tensor.matmul(
                                y[:],
                                h2[:, hh, p * 128 : (p + 1) * 128],
                                W2[:, hh, :],
                                start=(hh == 0),
                                stop=False,
                            )
                    nc.tensor.matmul(
                        y[:], ones1[:], b2r[:], start=False, stop=False,
                    )
                    # out = xn + y via identity matmul, then ACT drains the
                    # PSUM as f16 for the store (host upcasts to f32)
                    nc.tensor.matmul(
                        y[:], idn[:], xn[:, p, :], start=False, stop=True,
                    )
                    ot = op_.tile([128, H], f16, name=f"o_{ci}_{p}", tag="o")
                    nc.scalar.activation(ot[:], y[:], AF.Copy)
                    r0 = t0 - WARM + p * 128
                    nc.sync.dma_start(out_e[r0 : r0 + 128, :], ot[:])

            # software pipeline.  Issue order per iteration: front(ci),
            # back(ci-1), scan(ci).  The scans go LAST: they wait on the
            # entire front chain (transpose -> cast -> matmul -> sigmoid ->
            # gate math), and issuing them before back(ci-1) would block
            # the ready back-work behind that latency on the in-order DVE.
            # (A 4-stage variant with the FFN a further chunk behind
            # measured 290-320us vs 274us for this order.)
            st_prev = None
            for ci in range(N_CHUNKS + 1):
                fr = front(ci)
                if st_prev is not None:
                    back_gh(back_ef(st_prev))
                st_prev = scan_stage(fr)
            back_gh(back_ef(st_prev))

    _split_excess_waits(nc)
    return nc


def _prep_inputs(x, ln1_g, ln1_b, Wz, bz, Wh, bh, ln2_g, ln2_b, W1, b1, W2, b2):
    """Fold LN affine params into weights; build per-core input maps."""
    import ml_dtypes

    f32 = np.float32
    f8np = ml_dtypes.float8_e4m3
    Wzf = (ln1_g[:, None] * Wz).astype(f32)
    bzf = (bz + ln1_b @ Wz).astype(f32)
    Whf = (ln1_g[:, None] * Wh).astype(f32)
    bhf = (bh + ln1_b @ Wh).astype(f32)
    W1f = (ln2_g[:, None] * W1).astype(f32)
    b1f = (b1 + ln2_b @ W1).astype(f32)

    def wpack(w, dt):
        # [H, H] -> [128, HC*H]: plane hi holds contraction rows hi*128..
        return np.ascontiguousarray(
            w.reshape(H // 128, 128, H).transpose(1, 0, 2).reshape(128, -1)
        ).astype(dt)

    wz8 = wpack(Wzf, f8np if FP8_WZ else np.float16)
    wh8 = wpack(Whf, np.float16)
    w18 = wpack(W1f, f8np if FP8_FFN1 else np.float16)
    w28 = wpack(np.asarray(W2, f32), f8np if FP8_FFN2 else np.float16)

    b2f = np.asarray(b2, f32)
    if FP8_FFN2:
        # fold the mean FFN2 weight-quantization error into the bias:
        # E[h2_i] ~ E[relu(N(b1_i, s_i))] with s_i = ||W1f[:,i]|| (u2 is
        # LN-normalized), so E[dy] = Eh2 @ (q(W2) - W2) is input-free.
        import math

        s = np.linalg.norm(W1f, axis=0) + 1e-12
        r = (b1f / s).astype(np.float64)
        phi = np.exp(-0.5 * r * r) / math.sqrt(2 * math.pi)
        Phi = 0.5 * (1.0 + np.vectorize(math.erf)(r / math.sqrt(2)))
        eh2 = (s * phi + b1f * Phi).astype(f32)
        dW2 = w28.astype(f32) - wpack(np.asarray(W2, f32), f32)
        # eh2 must be permuted to match wpack's row layout: row p of the
        # packed tile, plane k -> original row k*128+p
        eh2p = eh2.reshape(H // 128, 128).transpose(1, 0)  # [128, HC]
        corr = np.einsum("pk,pkm->m", eh2p, dW2.reshape(128, H // 128, H))
        b2f = b2f - corr.astype(f32)
    b2r = b2f.astype(np.float16).reshape(1, H)

    def pack_mi(m, c):
        cols = []
        for vec in (bzf, bhf, bhf + 0.5, b1f):
            for hc in range(H // 128):
                cols.append(vec[hc * 128 : (hc + 1) * 128])
        cols.append(np.full(128, m, f32))
        cols.append(np.full(128, c, f32))
        for hc in range(H // 128):
            cols.append(-bzf[hc * 128 : (hc + 1) * 128])
        return np.stack(cols, axis=1).astype(f32)

    mi0 = pack_mi(0.0, 0.5)
    mi1 = pack_mi(1.0, 0.0)
    idn = np.eye(128, dtype=np.float16)

    in_maps = []
    for core in range(N_CORES):
        b, half = divmod(core, 2)
        if half == 0:
            xsrc = np.concatenate([x[b, 0:WARM], x[b, 0:HALF_T]], axis=0)
            mi = mi0
        else:
            xsrc = np.concatenate(
                [x[b, HALF_T - WARM : HALF_T], x[b, HALF_T:T]], axis=0
            )
            mi = mi1
        in_maps.append(
            {
                "xs": np.ascontiguousarray(xsrc).astype(np.float16),
                "wz": wz8,
                "wh": wh8,
                "w1": w18,
                "w2": w28,
                "mi": mi,
                "b2": b2r,
                "idn": idn,
            }
        )
    return in_maps


def run(in_maps, **kw):
    from concourse.bass_utils import run_bass_kernel_spmd

    if "nc" not in _cache:
        _cache["nc"] = _build()
    return run_bass_kernel_spmd(_cache["nc"], in_maps, list(range(N_CORES)), **kw)


def kernel(**inputs):
    inputs = {k: np.asarray(v) for k, v in inputs.items()}
    in_maps = _prep_inputs(**inputs)
    res = run(in_maps)
    out = np.empty((B, T, H), np.float32)
    for core in range(N_CORES):
        b, half = divmod(core, 2)
        out[b, half * HALF_T : (half + 1) * HALF_T] = res.results[core]["out"].astype(
            np.float32
        )
    return out


# revision 65
# speedup vs baseline: 1.0112x; 1.0112x over previous
"""MinGRU block kernel for 8 TRN2 NeuronCores.

Sharding: core c -> (batch b = c//2, T-half = c%2).  Each core processes
4096 rows of (T=8192) for one batch plus a 128-row scan warmup prefix.
The minGRU recurrence h_t = (1-z_t) h_{t-1} + z_t g_t is evaluated in
linear space with the DVE TensorTensorScan instruction (state fp32);
the warmup prefix exploits exponential forgetting (prod(1-z) < e^-30
over 128 steps) so no cross-core communication is needed: the half=1
core recomputes its predecessor's last 128 rows, the half=0 core scans
128 masked dummy rows and blends its true initial state (0.5) instead.

Engine placement per 512-row chunk (trace-tuned):
  PE   : gate-z + FFN2 matmuls in fp8e4 DoubleRow (2 accum steps of
         K=256), gate-p + FFN1 in fp16 (accuracy), plus rank-1 b2-bias
         and rank-128 identity matmuls that add the residual in PSUM.
  DVE  : bn_stats/aggr + rsqrt Newton seed, LN affine applies, a=1-z,
         g=max(p+bh+.5,s), b=g*z, xn residual add, the 4 scans.
  ACT  : sigmoids, relu, the uT fp16->fp8 cast, psY f32->f16 drain.
  SP   : every DMA: x loads, uT/u2T/hT xbar transposes, out stores.
  Pool : nothing — the TRN2 Pool engine cannot touch PSUM and lacks
         TensorScalarPtr, and its 0.42-efficiency TensorTensor ops
         lose more in scan-chain latency than they save on DVE.
PSUM: 4 banks gate psums, 4 banks FFN1/FFN2+residual (shared ring).
LayerNorm gains/biases are folded into the weight matrices host-side;
x is fp16 host-side (halves input DMA); out is stored f16 and upcast
on the host (error budget is 2e-2 of max|out| ~ 0.196 abs).
"""

import numpy as np

B, T, H = 4, 8192, 512
LN_EPS = 1e-5
HALF_T = T // 2          # rows per core (output)
WARM = 128               # scan warmup rows
ROWS = HALF_T + WARM     # input rows per core
N_CORES = 8
CHUNK = 512              # rows per pipeline chunk
N_CHUNKS = HALF_T // CHUNK
HC = H // 128            # 4 H-chunks

# precision config: each fp8 matmul path adds ~0.13-0.15 max-abs error of
# the 0.196 budget (2e-2 of max|out|), combining in quadrature.  fp8 for
# the z-gate (sigmoid compresses 4x) and FFN2 (with host-side bias feedback
# of the mean weight-quantization error) measures ~1.6e-2; Wh and FFN1
# stay fp16.
FP8_WZ = True
FP8_FFN1 = False
FP8_FFN2 = True
SCAN_ON_POOL = 0         # Pool lacks TensorScalarPtr on trn2: scans stay DVE
NEWTON_ITERS = 1

_cache = {}


# ---------------------------------------------------------------------------
# walrus workaround: the compiler in this container caps sync commands per
# instruction at 1 wait + 1 update.  Tile attaches N waits/updates freely;
# split the excess onto same-engine NoOps (before for waits, after for
# updates).
# ---------------------------------------------------------------------------
def _split_excess_waits(nc):
    import bass_rust

    ctr = [0]

    def mknop(engine, waits, updates):
        ctr[0] += 1
        nop = bass_rust.InstNoOp(name=f"splitw-{ctr[0]}")
        nop.engine = engine
        nop.sync_info = bass_rust.SyncInfo(on_wait=list(waits), on_update=list(updates))
        nc.register_instruction(nop)
        return nop

    for f in nc.m.functions:
        for bb in f.blocks:
            insts = list(bb.instructions)
            out = []
            changed = False
            for ins in insts:
                si = ins.sync_info
                if si is None:
                    out.append(ins)
                    continue
                waits = list(si.on_wait or [])
                updates = list(si.on_update or [])
                if len(waits) <= 1 and len(updates) <= 1:
                    out.append(ins)
                    continue
                changed = True
                for w in waits[1:]:
                    out.append(mknop(ins.engine, [w], []))
                si.on_wait = waits[:1]
                si.on_update = updates[:1]
                out.append(ins)
                for u in updates[1:]:
                    out.append(mknop(ins.engine, [], [u]))
            if changed:
                bb.instructions = out


# ---------------------------------------------------------------------------
# kernel builder
# ---------------------------------------------------------------------------
def _build():
    import concourse.bass as bass
    import concourse.tile as tile
    from concourse import mybir

    f32, f16 = mybir.dt.float32, mybir.dt.float16
    f8 = mybir.dt.float8e4
    AF = mybir.ActivationFunctionType
    OP = mybir.AluOpType
    DR = mybir.MatmulPerfMode.DoubleRow

    zdt = f8 if FP8_WZ else f16
    w1dt = f8 if FP8_FFN1 else f16
    w2dt = f8 if FP8_FFN2 else f16

    nc = bass.Bass()
    xs_e = nc.declare_dram_parameter("xs", [ROWS, H], f16, isOutput=False)
    wz_e = nc.declare_dram_parameter("wz", [128, HC * H], zdt, isOutput=False)
    wh_e = nc.declare_dram_parameter("wh", [128, HC * H], f16, isOutput=False)
    w1_e = nc.declare_dram_parameter("w1", [128, HC * H], w1dt, isOutput=False)
    w2_e = nc.declare_dram_parameter("w2", [128, HC * H], w2dt, isOutput=False)
    # packed per-partition scalars: cols 0-3 bz, 4-7 bh, 8-11 bh+0.5,
    # 12-15 b1 (per 128-channel chunk), 16 m (carry mask), 17 c (carry bias)
    mi_e = nc.declare_dram_parameter("mi", [128, 22], f32, isOutput=False)
    b2_e = nc.declare_dram_parameter("b2", [1, H], f16, isOutput=False)
    id_e = nc.declare_dram_parameter("idn", [128, 128], f16, isOutput=False)
    out_e = nc.declare_dram_parameter("out", [HALF_T, H], f16, isOutput=True)

    with tile.TileContext(nc) as tc:
        from contextlib import ExitStack

        with ExitStack() as ctx:
            ep = ctx.enter_context

            const = ep(tc.tile_pool(name="const", bufs=1))
            xp = ep(tc.tile_pool(name="xp", bufs=4))
            up = ep(tc.tile_pool(name="up", bufs=4))
            uTp = ep(tc.tile_pool(name="uTp", bufs=4))
            u8p = ep(tc.tile_pool(name="u8p", bufs=4))
            gp = ep(tc.tile_pool(name="gp", bufs=8))
            hp = ep(tc.tile_pool(name="hp", bufs=10))
            xnp = ep(tc.tile_pool(name="xnp", bufs=4))
            hnp = ep(tc.tile_pool(name="hnp", bufs=3))
            h2p = ep(tc.tile_pool(name="h2p", bufs=4))
            op_ = ep(tc.tile_pool(name="op", bufs=4))
            stp = ep(tc.tile_pool(name="stp", bufs=24))
            # 8 PSUM banks: gates 4, FFN1+FFN2 share 4 (same tile shape);
            # the h-transpose runs on the DMA xbar so no PSUM is needed
            psG = ep(tc.tile_pool(name="psG", bufs=4, space="PSUM"))
            psFY = ep(tc.tile_pool(name="psFY", bufs=4, space="PSUM"))

            # ---- constants ----
            def load_w(name, ext, dt):
                t = const.tile([128, HC * H], dt, name=name, tag=name)
                nc.sync.dma_start(t[:], ext[:])
                # [128, hi, m] view: hi = contraction plane, m = out channel
                return t[:].rearrange("a (k m) -> a k m", k=HC)

            WZ = load_w("wz", wz_e, zdt)
            WH = load_w("wh", wh_e, f16)
            W1 = load_w("w1", w1_e, w1dt)
            W2 = load_w("w2", w2_e, w2dt)

            mi = const.tile([128, 22], f32, name="mi", tag="mi")
            nc.sync.dma_start(mi[:], mi_e[:])
            b2r = const.tile([1, H], f16, name="b2r", tag="b2r")
            nc.sync.dma_start(b2r[:], b2_e[:])
            ones1 = const.tile([1, 128], f16, name="ones1", tag="ones1")
            nc.gpsimd.memset(ones1[:], 1.0)
            idn = const.tile([128, 128], f16, name="idn", tag="idn")
            nc.sync.dma_start(idn[:], id_e[:])

            BZ = [mi[:, j : j + 1] for j in range(0, 4)]
            BH = [mi[:, j : j + 1] for j in range(4, 8)]
            BH05 = [mi[:, j : j + 1] for j in range(8, 12)]
            B1 = [mi[:, j : j + 1] for j in range(12, 16)]
            M_AP = mi[:, 16:17]
            C_AP = mi[:, 17:18]

            i32 = mybir.dt.int32

            def rstd_and_nm(mvall, n, ci, which):
                """mvall [128, 2n] = (means | vars) -> (rstd y, -mu*rstd nm).

                rstd = 1/sqrt(var+eps): q=1/(var+eps) (HW divide), sqrt
                bit-hack seed (i>>1)+0x1fbd1df5, Newton rsqrt steps.
                """
                means, vars_ = mvall[:, 0:n], mvall[:, n : 2 * n]
                ve = stp.tile([128, n], f32, name=f"ve{which}_{ci}", tag="ve")
                nc.vector.tensor_scalar(ve[:], vars_, LN_EPS, None, OP.add)
                q = stp.tile([128, n], f32, name=f"q{which}_{ci}", tag="q")
                nc.vector.reciprocal(q[:], ve[:])
                y = stp.tile([128, n], f32, name=f"y{which}_{ci}", tag="y")
                nc.vector.tensor_scalar(
                    y[:].bitcast(i32), q[:].bitcast(i32), 1, None,
                    OP.logical_shift_right,
                )
                nc.vector.tensor_scalar(
                    y[:].bitcast(i32), y[:].bitcast(i32), 0x1FBD1DF5, None, OP.add
                )
                w = stp.tile([128, n], f32, name=f"w{which}_{ci}", tag="w")
                for _ in range(NEWTON_ITERS):  # y <- y*(1.5 - 0.5*ve*y^2)
                    nc.vector.tensor_mul(w[:], y[:], y[:])
                    nc.vector.tensor_mul(w[:], w[:], ve[:])
                    nc.vector.tensor_scalar(w[:], w[:], -0.5, 1.5, OP.mult, OP.add)
                    nc.vector.tensor_mul(y[:], y[:], w[:])
                nm = stp.tile([128, n], f32, name=f"nm{which}_{ci}", tag="nm")
                nc.vector.scalar_tensor_tensor(nm[:], means, -1.0, y[:], OP.mult, OP.mult)
                return y, nm

            def layernorm_group(src, nsub, ci, which, pool, tag, tbufs=None, on_act=False):
                """standardize rows of src [128, nsub, H] -> f16 tile same shape.

                bn_stats/aggr on DVE for the stats; the affine apply is a
                DVE tensor_scalar (4x fast mode) with per-partition scale
                rstd and bias -mu*rstd.
                """
                n = nsub
                mvall = stp.tile([128, 2 * n], f32, name=f"mv{which}_{ci}", tag="mv")
                for p in range(n):
                    st = stp.tile([128, 6], f32, name=f"bn{which}_{ci}_{p}", tag="bn")
                    nc.vector.bn_stats(st[:], src[:, p, :])
                    # mean -> col p, var -> col n+p  (stride-n pair)
                    nc.vector.bn_aggr(mvall[:, p : p + n + 1 : n], st[:])
                y, nm = rstd_and_nm(mvall[:], n, ci, which)
                ut = pool.tile(
                    [128, nsub, H], f16, name=f"{tag}_{ci}", tag=tag, bufs=tbufs
                )
                for p in range(n):
                    if on_act:
                        nc.scalar.activation(
                            ut[:, p, :], src[:, p, :], AF.Identity,
                            bias=nm[:, p : p + 1], scale=y[:, p : p + 1],
                        )
                    else:
                        nc.vector.tensor_scalar(
                            ut[:, p, :], src[:, p, :],
                            y[:, p : p + 1], nm[:, p : p + 1],
                            OP.mult, OP.add,
                        )
                return ut

            def transpose_cast(ut, nsub, tlen, ci, tag, cast, tbufs=None, eng=None):
                """u [128, nsub, H] f16 -> uT [128, HC, tlen] (+fp8 copy).

                DMA xbar transpose per subtile into an f16 tile, then one
                ACT copy casts the whole tile to fp8 for DoubleRow matmuls.
                """
                tT = uTp.tile(
                    [128, HC, tlen], f16, name=f"{tag}T_{ci}", tag=f"{tag}T", bufs=tbufs
                )
                for p in range(nsub):
                    (eng or nc.sync).dma_start_transpose(
                        tT[:, :, p * 128 : (p + 1) * 128], ut[:, p, :]
                    )
                if not cast:
                    return tT, tT
                t8 = u8p.tile(
                    [128, HC, tlen], f8, name=f"{tag}8_{ci}", tag=f"{tag}8", bufs=tbufs
                )
                nc.scalar.activation(
                    t8[:].rearrange("a k m -> a (k m)"),
                    tT[:].rearrange("a k m -> a (k m)"),
                    AF.Copy,
                )
                return tT, t8

            def mm_accum(ps, W, rhs8, ho, fp8):
                """accumulate ps[:, :tlen] = (W^T @ u)[ho block] over K=512."""
                if fp8:
                    for ki in range(HC // 2):
                        nc.tensor.matmul(
                            ps,
                            W[:, 2 * ki : 2 * ki + 2, ho * 128 : (ho + 1) * 128],
                            rhs8[:, 2 * ki : 2 * ki + 2, :],
                            start=(ki == 0),
                            stop=(ki == HC // 2 - 1),
                            perf_mode=DR,
                        )
                else:
                    for hi in range(HC):
                        nc.tensor.matmul(
                            ps,
                            W[:, hi, ho * 128 : (ho + 1) * 128],
                            rhs8[:, hi, :],
                            start=(hi == 0),
                            stop=(hi == HC - 1),
                        )

            carry = [None] * HC  # AP of [128,1] initial state per H-chunk

            def front(ci):
                warm = ci == 0
                tlen = WARM if warm else CHUNK
                t0 = 0 if warm else WARM + (ci - 1) * CHUNK
                nsub = tlen // 128
                wtag = "w" if warm else ""
                wb = 1 if warm else None

                # ---- stage A: load + LN1 ----
                xt = xp.tile([128, nsub, H], f16, name=f"x_{ci}", tag=f"x{wtag}", bufs=wb)
                nc.sync.dma_start(
                    xt[:],
                    xs_e[t0 : t0 + tlen, :].rearrange("(s p) h -> p s h", p=128),
                )
                ut = layernorm_group(xt, nsub, ci, 1, up, f"u{wtag}", wb)
                uT, u8 = transpose_cast(ut, nsub, tlen, ci, f"u{wtag}", FP8_WZ, wb)

                # ---- stage B+C: gate matmuls, gates ----
                # pT first: its consumer chain (s -> g -> b, through 3
                # engines) is the long pole into the scan; kT -> z -> a is
                # short and also needs the fp8 cast.
                gates = []
                for ho in range(HC):
                    pTt = psG.tile([128, CHUNK], f32, name=f"pT_{ci}_{ho}", tag="psG")
                    pT = pTt[:, :tlen]
                    mm_accum(pT, WH, uT, ho, False)
                    kTt = psG.tile([128, CHUNK], f32, name=f"kT_{ci}_{ho}", tag="psG")
                    kT = kTt[:, :tlen]
                    mm_accum(kT, WZ, u8, ho, FP8_WZ)
                    s = gp.tile([128, tlen], f16, name=f"s_{ci}_{ho}", tag=f"s{wtag}", bufs=wb)
                    nc.scalar.activation(s[:], pT, AF.Sigmoid, bias=BH[ho], scale=1.0)
                    g = gp.tile([128, tlen], f16, name=f"g_{ci}_{ho}", tag=f"g{wtag}", bufs=wb)
                    nc.vector.scalar_tensor_tensor(
                        g[:], pT, BH05[ho], s[:], OP.add, OP.max
                    )
                    z = gp.tile([128, tlen], f16, name=f"z_{ci}_{ho}", tag=f"z{wtag}", bufs=wb)
                    nc.scalar.activation(z[:], kT, AF.Sigmoid, bias=BZ[ho], scale=1.0)
                    a = gp.tile([128, tlen], f16, name=f"a_{ci}_{ho}", tag=f"a{wtag}", bufs=wb)
                    nc.vector.tensor_scalar(a[:], z[:], -1.0, 1.0, OP.mult, OP.add)
                    b = gp.tile([128, tlen], f16, name=f"b_{ci}_{ho}", tag=f"b{wtag}", bufs=wb)
                    nc.vector.tensor_mul(b[:], g[:], z[:])
                    gates.append((a, b))

                return ci, warm, tlen, xt, gates

            def scan_stage(st):
                ci, warm, tlen, xt, gates = st
                wtag = "w" if warm else ""
                wb = 1 if warm else None
                hTs = []
                for ho in range(HC):
                    a, b = gates[ho]
                    hT = hp.tile(
                        [128, tlen], f16, name=f"hT_{ci}_{ho}", tag=f"hT{wtag}", bufs=wb
                    )
                    init = 0.5 if warm else carry[ho]
                    eng = nc.gpsimd if ho >= HC - SCAN_ON_POOL else nc.vector
                    eng.tensor_tensor_scan(
                        hT[:], a[:], b[:], init, OP.mult, OP.add
                    )
                    hTs.append(hT)

                if warm:
                    # blend: init = m * h_warm_end + c   (m=0 -> 0.5, m=1 -> carry)
                    for ho in range(HC):
                        bl = stp.tile([128, 1], f32, name=f"bl_{ho}", tag="bl")
                        nc.vector.scalar_tensor_tensor(
                            bl[:],
                            hTs[ho][:, tlen - 1 : tlen],
                            M_AP,
                            C_AP,
                            OP.mult,
                            OP.add,
                        )
                        carry[ho] = bl[:]
                    return None

                for ho in range(HC):
                    carry[ho] = hTs[ho][:, tlen - 1 : tlen]
                return ci, xt, hTs

            def back_ef(state):
                ci, xt, hTs = state
                tlen = CHUNK
                nsub = tlen // 128

                # ---- stage E: h back to natural (DMA xbar transpose) ----
                hn = hnp.tile([128, nsub, H], f16, name=f"hN_{ci}", tag="hN")
                for hc in range(HC):
                    nc.sync.dma_start_transpose(
                        hn[:].rearrange("a s h -> a s h")[
                            :, :, hc * 128 : (hc + 1) * 128
                        ],
                        hTs[hc][:],
                    )
                xn = xnp.tile([128, nsub, H], f16, name=f"xn_{ci}", tag="xn")
                for p in range(nsub):
                    nc.vector.tensor_add(xn[:, p, :], xt[:, p, :], hn[:, p, :])
                # ---- stage F: LN2 ----
                u2 = layernorm_group(xn, nsub, ci, 2, up, "u2")
                _, u28 = transpose_cast(u2, nsub, tlen, ci, "u2", FP8_FFN1)
                return ci, xn, u28

            def back_gh(state):
                ci, xn, u28 = state
                tlen = CHUNK
                t0 = WARM + (ci - 1) * CHUNK
                nsub = tlen // 128

                # ---- stage G: FFN1 + relu ----
                h2 = h2p.tile([128, HC, tlen], w2dt, name=f"h2_{ci}", tag="h2")
                for hh in range(HC):
                    h1 = psFY.tile([128, tlen], f32, name=f"h1_{ci}_{hh}", tag="psFY")
                    mm_accum(h1[:], W1, u28, hh, FP8_FFN1)
                    nc.scalar.activation(
                        h2[:, hh, :], h1[:], AF.Relu, bias=B1[hh], scale=1.0
                    )

                # ---- stage H: FFN2 + bias + residual (all in PSUM) + store ----
                for p in range(nsub):
                    y = psFY.tile([128, H], f32, name=f"y_{ci}_{p}", tag="psFY")
                    if FP8_FFN2:
                        for ki in range(HC // 2):
                            nc.tensor.matmul(
                                y[:],
                                h2[:, 2 * ki : 2 * ki + 2, p * 128 : (p + 1) * 128],
                                W2[:, 2 * ki : 2 * ki + 2, :],
                                start=(ki == 0),
                                stop=False,
                                perf_mode=DR,
                            )
                    else:
                        for hh in range(HC):
                            nc.tensor.matmul(
                                y[:],
                                h2[:, hh, p * 128 : (p + 1) * 128],
                                W2[:, hh, :],
                                start=(hh == 0),
                                stop=False,
                            )
                    nc.tensor.matmul(
                        y[:], ones1[:], b2r[:], start=False, stop=False,
                    )
                    # out = xn + y via identity matmul, then ACT drains the
                    # PSUM as f16 for the store (host upcasts to f32)
                    nc.tensor.matmul(
                        y[:], idn[:], xn[:, p, :], start=False, stop=True,
                    )
                    ot = op_.tile([128, H], f16, name=f"o_{ci}_{p}", tag="o")
                    nc.scalar.activation(ot[:], y[:], AF.Copy)
                    r0 = t0 - WARM + p * 128
                    nc.sync.dma_start(out_e[r0 : r0 + 128, :], ot[:])

            # software pipeline.  Issue order per iteration: front(ci),
            # back(ci-1), scan(ci).  The scans go LAST: they wait on the
            # entire front chain (transpose -> cast -> matmul -> sigmoid ->
            # gate math), and issuing them before back(ci-1) would block
            # the ready back-work behind that latency on the in-order DVE.
            # (A 4-stage variant with the FFN a further chunk behind
            # measured 290-320us vs 274us for this order.)
            st_prev = None
            for ci in range(N_CHUNKS + 1):
                fr = front(ci)
                if st_prev is not None:
                    back_gh(back_ef(st_prev))
                st_prev = scan_stage(fr)
            back_gh(back_ef(st_prev))

    _split_excess_waits(nc)
    return nc


def _prep_inputs(x, ln1_g, ln1_b, Wz, bz, Wh, bh, ln2_g, ln2_b, W1, b1, W2, b2):
    """Fold LN affine params into weights; build per-core input maps."""
    import ml_dtypes

    f32 = np.float32
    f8np = ml_dtypes.float8_e4m3
    Wzf = (ln1_g[:, None] * Wz).astype(f32)
    bzf = (bz + ln1_b @ Wz).astype(f32)
    Whf = (ln1_g[:, None] * Wh).astype(f32)
    bhf = (bh + ln1_b @ Wh).astype(f32)
    W1f = (ln2_g[:, None] * W1).astype(f32)
    b1f = (b1 + ln2_b @ W1).astype(f32)

    def wpack(w, dt):
        # [H, H] -> [128, HC*H]: plane hi holds contraction rows hi*128..
        return np.ascontiguousarray(
            w.reshape(H // 128, 128, H).transpose(1, 0, 2).reshape(128, -1)
        ).astype(dt)

    wz8 = wpack(Wzf, f8np if FP8_WZ else np.float16)
    wh8 = wpack(Whf, np.float16)
    w18 = wpack(W1f, f8np if FP8_FFN1 else np.float16)
    w28 = wpack(np.asarray(W2, f32), f8np if FP8_FFN2 else np.float16)

    b2f = np.asarray(b2, f32)
    if FP8_FFN2:
        # fold the mean FFN2 weight-quantization error into the bias:
        # E[h2_i] ~ E[relu(N(b1_i, s_i))] with s_i = ||W1f[:,i]|| (u2 is
        # LN-normalized), so E[dy] = Eh2 @ (q(W2) - W2) is input-free.
        import math

        s = np.linalg.norm(W1f, axis=0) + 1e-12
        r = (b1f / s).astype(np.float64)
        phi = np.exp(-0.5 * r * r) / math.sqrt(2 * math.pi)
        Phi = 0.5 * (1.0 + np.vectorize(math.erf)(r / math.sqrt(2)))
        eh2 = (s * phi + b1f * Phi).astype(f32)
        dW2 = w28.astype(f32) - wpack(np.asarray(W2, f32), f32)
        # eh2 must be permuted to match wpack's row layout: row p of the
        # packed tile, plane k -> original row k*128+p
        eh2p = eh2.reshape(H // 128, 128).transpose(1, 0)  # [128, HC]
        corr = np.einsum("pk,pkm->m", eh2p, dW2.reshape(128, H // 128, H))
        b2f = b2f - corr.astype(f32)
    b2r = b2f.astype(np.float16).reshape(1, H)

    def pack_mi(m, c):
        cols = []
        for vec in (bzf, bhf, bhf + 0.5, b1f):
            for hc in range(H // 128):
                cols.append(vec[hc * 128 : (hc + 1) * 128])
        cols.append(np.full(128, m, f32))
        cols.append(np.full(128, c, f32))
        for hc in range(H // 128):
            cols.append(-bzf[hc * 128 : (hc + 1) * 128])
        return np.stack(cols, axis=1).astype(f32)

    mi0 = pack_mi(0.0, 0.5)
    mi1 = pack_mi(1.0, 0.0)
    idn = np.eye(128, dtype=np.float16)

    in_maps = []
    for core in range(N_CORES):
        b, half = divmod(core, 2)
        if half == 0:
            xsrc = np.concatenate([x[b, 0:WARM], x[b, 0:HALF_T]], axis=0)
            mi = mi0
        else:
            xsrc = np.concatenate(
                [x[b, HALF_T - WARM : HALF_T], x[b, HALF_T:T]], axis=0
            )
            mi = mi1
        in_maps.append(
            {
                "xs": np.ascontiguousarray(xsrc).astype(np.float16),
                "wz": wz8,
                "wh": wh8,
                "w1": w18,
                "w2": w28,
                "mi": mi,
                "b2": b2r,
                "idn": idn,
            }
        )
    return in_maps


def run(in_maps, **kw):
    from concourse.bass_utils import run_bass_kernel_spmd

    if "nc" not in _cache:
        _cache["nc"] = _build()
    return run_bass_kernel_spmd(_cache["nc"], in_maps, list(range(N_CORES)), **kw)


def kernel(**inputs):
    inputs = {k: np.asarray(v) for k, v in inputs.items()}
    in_maps = _prep_inputs(**inputs)
    res = run(in_maps)
    out = np.empty((B, T, H), np.float32)
    for core in range(N_CORES):
        b, half = divmod(core, 2)
        out[b, half * HALF_T : (half + 1) * HALF_T] = res.results[core]["out"].astype(
            np.float32
        )
    return out
